# revision 25
# baseline (speedup 1.0000x reference)
"""Trainium2 Bass kernel for nn_AttnCalc (additive attention with coverage).

Math (see reference):
  B, L, H = 16, 96, 128
  enc_f[b,o,w] = conv2d(encoder_outputs as NCHW [B,L,1,H], W[L,L,H,H], same pad)
               = sum_{i,s'} W[o,i,63,s'] * x[b,i,w+s'-63]        (only kh=63 row survives)
  cvg_f[b,o]   = sum_i cvg_w[o,i,0,63] * coverage[b,i]           (only kw=63 col survives)
  dec_f[b,h]   = hidden @ dec_w.T + dec_b
  feats = tanh(enc_f + dec_f + cvg_f + enc_bias + cvg_bias)
  scores[b,l] = sum_h feats[b,l,h] * v[b,h];  attn = softmax_L(scores)
  context[b,h] = sum_l attn[b,l] * encoder_outputs[b,l,h]
  new_coverage = coverage + attn

Strategy: data-parallel over batch (2 per core, 8 cores, no collectives).
Per-core layout keeps L on SBUF partitions and (b, h/w) on the free dim:
the conv becomes 128 shift-matmuls accumulating into one PSUM tile
[96, 2*128], in float32r (TF32-class; measured max rel err ~1.5e-4).
"""

import os
import sys
import types

import numpy as np

sys.path.insert(0, "/opt/trn_rl_repo")

import concourse.bass as bass  # noqa: E402
import concourse.mybir as mybir  # noqa: E402
import concourse.tile as tile  # noqa: E402
from concourse import bacc  # noqa: E402
from concourse.bass_utils import run_bass_kernel_spmd  # noqa: E402

B, L, H = 16, 96, 128
NCORES = 8
BPC = B // NCORES          # batches per core
NPAD = 2 * H               # padded width (63 left + 128 + 65 right)
NCHUNK = 16                # weight DMA chunks
SPC = H // NCHUNK          # shifts per weight chunk

F32 = mybir.dt.float32
MM_DT_NAME = os.environ.get("MM_DT", "f32r")
F32R = {"f32r": mybir.dt.float32r,
        "bf16": mybir.dt.bfloat16,
        "f32": mybir.dt.float32}[MM_DT_NAME]

# results of the last traced run (read by test.py)
LAST_RESULTS = None


def _install_ntff_shim():
    """antenv.axon_hooks is missing from this image; recreate it so
    run_bass_kernel_spmd(trace=True) can capture NTFF profiles."""
    if "antenv.axon_hooks" in sys.modules:
        return
    try:
        import antenv
    except ImportError:
        return
    mod = types.ModuleType("antenv.axon_hooks")
    mod._hook = None

    def set_axon_ntff_profile_hook(h):
        mod._hook = h

    def get_axon_ntff_profile_hook():
        return mod._hook

    mod.set_axon_ntff_profile_hook = set_axon_ntff_profile_hook
    mod.get_axon_ntff_profile_hook = get_axon_ntff_profile_hook
    sys.modules["antenv.axon_hooks"] = mod
    antenv.axon_hooks = mod
    try:
        from trn_agent_boot.trn_boot import _ntff_profile_via_ctypes

        set_axon_ntff_profile_hook(
            _ntff_profile_via_ctypes("/opt/axon/libaxon_pjrt.so")
        )
    except Exception:
        pass


def _build():
    nc = bacc.Bacc("TRN2", target_bir_lowering=False, debug=False)

    # weights/constants shared by all cores (each core gets the same array)
    w_d = nc.dram_tensor("w", [L, H, L], F32R, kind="ExternalInput")      # [i, s', o]
    wc_d = nc.dram_tensor("wc", [L, L], F32R, kind="ExternalInput")       # [i, o]
    dwt_d = nc.dram_tensor("dwt", [H, H], F32, kind="ExternalInput")      # dec_w.T [k, h]
    db_d = nc.dram_tensor("db", [H, H], F32, kind="ExternalInput")        # dec_b/H tiled [k, h]
    bia_d = nc.dram_tensor("bia", [L, 2], F32, kind="ExternalInput")      # attn_b | cvg_b

    # per-core activations
    xp_d = nc.dram_tensor("xp", [L, BPC, NPAD], F32R, kind="ExternalInput")
    cb_d = nc.dram_tensor("cb", [L, BPC, H], F32R, kind="ExternalInput")  # cov bcast
    ct_d = nc.dram_tensor("ct", [L, BPC], F32, kind="ExternalInput")      # coverage.T
    ht_d = nc.dram_tensor("ht", [H, BPC], F32, kind="ExternalInput")      # hidden.T
    vb_d = nc.dram_tensor("vb", [L, BPC, H], F32, kind="ExternalInput")   # v bcast

    at_o = nc.dram_tensor("attn_t", [L, BPC], F32, kind="ExternalOutput")
    nc_o = nc.dram_tensor("ncov_t", [L, BPC], F32, kind="ExternalOutput")
    cx_o = nc.dram_tensor("ctx", [BPC, H], F32, kind="ExternalOutput")

    NF = BPC * H  # psum free size

    with tile.TileContext(nc) as tc:
        with (
            tc.tile_pool(name="pool", bufs=1) as pool,
            tc.tile_pool(name="psum", bufs=1, space="PSUM") as pp,
        ):
            # --- loads: small inputs first, then the 4.7MB weight stream
            # spread across all engines' DMA queues so transfers overlap
            # with the PE shift-matmul stream.
            xp = pool.tile([L, BPC, NPAD], F32R)
            nc.sync.dma_start(xp[:], xp_d[:])
            cb = pool.tile([L, BPC, H], F32R)
            nc.scalar.dma_start(cb[:], cb_d[:])
            wc = pool.tile([L, L], F32R)
            nc.gpsimd.dma_start(wc[:], wc_d[:])
            dwt = pool.tile([H, H], F32)
            nc.gpsimd.dma_start(dwt[:], dwt_d[:])
            db = pool.tile([H, H], F32)
            nc.sync.dma_start(db[:], db_d[:])
            ht = pool.tile([H, BPC], F32)
            nc.gpsimd.dma_start(ht[:], ht_d[:])
            vb = pool.tile([L, BPC, H], F32)
            nc.scalar.dma_start(vb[:], vb_d[:])
            ct = pool.tile([L, BPC], F32)
            nc.scalar.dma_start(ct[:], ct_d[:])
            bia = pool.tile([L, 2], F32)
            nc.gpsimd.dma_start(bia[:], bia_d[:])

            dma_engines = [nc.sync, nc.scalar, nc.gpsimd]
            chunk_order = [63 // SPC] + [c for c in range(NCHUNK)
                                        if c != 63 // SPC]
            wch = {}
            for idx, c in enumerate(chunk_order):
                t = pool.tile([L, SPC, L], F32R, tag=f"w{c}")
                dma_engines[idx % len(dma_engines)].dma_start(
                    t[:], w_d[:, c * SPC:(c + 1) * SPC, :])
                wch[c] = t

            ones = pool.tile([H, L], F32)
            nc.vector.memset(ones[:], 1.0)

            # g[k, b, h] = dec_w.T[k, h] * hidden.T[k, b] + dec_b[h]/H
            g = pool.tile([H, BPC, H], F32)
            for b in range(BPC):
                nc.vector.scalar_tensor_tensor(
                    out=g[:, b, :], in0=dwt[:], scalar=ht[:, b:b + 1], in1=db[:],
                    op0=mybir.AluOpType.mult, op1=mybir.AluOpType.add,
                )

            # --- accumulate enc_f + cvg_f + dec_f in PSUM [L, BPC*H] ---
            # s'=63 first: it covers the full tile, so it carries start=True
            # and the rest can follow in weight-chunk streaming order.
            acc = pp.tile([L, BPC, H], F32)
            shift_order = [63] + [s for c in chunk_order
                                  for s in range(c * SPC, (c + 1) * SPC)
                                  if s != 63]
            for s in shift_order:
                nc.tensor.matmul(
                    acc[:],
                    wch[s // SPC][:, s % SPC, :],
                    xp[:, :, s:s + H],
                    start=(s == 63), stop=False,
                )
            nc.tensor.matmul(acc[:], wc[:], cb[:],
                             start=False, stop=False)
            nc.tensor.matmul(acc[:], ones[:], g[:],
                             start=False, stop=True)

            # --- epilogue ---
            tb = pool.tile([L, 1], F32)
            nc.vector.tensor_add(tb[:], bia[:, 0:1], bia[:, 1:2])
            feats = pool.tile([L, BPC, H], F32)
            nc.scalar.activation(
                feats[:], acc[:],
                mybir.ActivationFunctionType.Tanh, bias=tb[:], scale=1.0,
            )
            prod = pool.tile([L, BPC, H], F32)
            scores = pool.tile([L, BPC], F32)
            nc.vector.tensor_mul(prod[:], feats[:], vb[:])
            nc.vector.tensor_reduce(scores[:], prod[:],
                                    axis=mybir.AxisListType.X,
                                    op=mybir.AluOpType.add)
            e = pool.tile([L, BPC], F32)
            nc.scalar.activation(e[:], scores[:], mybir.ActivationFunctionType.Exp)

            sums = pp.tile([1, BPC], F32)
            nc.tensor.matmul(sums[:], ones[:L, 0:1], e[:], start=True, stop=True)
            recip = pool.tile([1, BPC], F32)
            nc.vector.reciprocal(recip[:], sums[:])
            recb = pool.tile([L, BPC], F32)
            nc.gpsimd.partition_broadcast(recb[:], recip[:])

            attn = pool.tile([L, BPC], F32)
            nc.vector.tensor_mul(attn[:], e[:], recb[:])
            ncov = pool.tile([L, BPC], F32)
            nc.vector.tensor_add(ncov[:], attn[:], ct[:])
            nc.sync.dma_start(at_o[:], attn[:])
            nc.sync.dma_start(nc_o[:], ncov[:])

            if MM_DT_NAME == "bf16":
                attn_mm = pool.tile([L, BPC], F32R)
                nc.vector.tensor_copy(attn_mm[:], attn[:])
            for b in range(BPC):
                cx = pp.tile([1, H], F32, tag=f"cx{b}")
                if MM_DT_NAME == "bf16":
                    nc.tensor.matmul(
                        cx[:], attn_mm[:, b:b + 1], xp[:, b, 63:63 + H],
                        start=True, stop=True,
                    )
                else:
                    nc.tensor.matmul(
                        cx[:], attn[:, b:b + 1],
                        xp[:, b, 63:63 + H].bitcast(F32),
                        start=True, stop=True,
                    )
                cxs = pool.tile([1, H], F32, tag=f"cxs{b}")
                nc.vector.tensor_copy(cxs[:], cx[:])
                nc.sync.dma_start(cx_o[b:b + 1, :], cxs[:])

    nc.compile()
    return nc


def _declare_drams(nc):
    d = {}
    d["w"] = nc.dram_tensor("w", [L, H, L], F32R, kind="ExternalInput")
    d["wc"] = nc.dram_tensor("wc", [L, L], F32R, kind="ExternalInput")
    d["dwt"] = nc.dram_tensor("dwt", [H, H], F32, kind="ExternalInput")
    d["db"] = nc.dram_tensor("db", [H, H], F32, kind="ExternalInput")
    d["bia"] = nc.dram_tensor("bia", [L, 2], F32, kind="ExternalInput")
    d["xp"] = nc.dram_tensor("xp", [L, BPC, NPAD], F32R, kind="ExternalInput")
    d["cb"] = nc.dram_tensor("cb", [L, BPC, H], F32R, kind="ExternalInput")
    d["ct"] = nc.dram_tensor("ct", [L, BPC], F32, kind="ExternalInput")
    d["ht"] = nc.dram_tensor("ht", [H, BPC], F32, kind="ExternalInput")
    d["vb"] = nc.dram_tensor("vb", [L, BPC, H], F32, kind="ExternalInput")
    d["attn_t"] = nc.dram_tensor("attn_t", [L, BPC], F32, kind="ExternalOutput")
    d["ncov_t"] = nc.dram_tensor("ncov_t", [L, BPC], F32, kind="ExternalOutput")
    d["ctx"] = nc.dram_tensor("ctx", [BPC, H], F32, kind="ExternalOutput")
    return d


def _build_raw():
    """Raw bacc (no Tile framework): hand-placed semaphores, no kernel-end
    barrier storm, DMA triggers issued from instruction 0."""
    from contextlib import ExitStack

    nc = bacc.Bacc("TRN2", target_bir_lowering=False, debug=False)
    d = _declare_drams(nc)

    with ExitStack() as ctx:
        def sb(name, shape, dt):
            return ctx.enter_context(nc.sbuf_tensor(name, list(shape), dt))

        def ps(name, shape, dt):
            return ctx.enter_context(nc.psum_tensor(name, list(shape), dt))

        def sem(name):
            return ctx.enter_context(nc.semaphore(name))

        wch = {c: sb(f"wch{c}", [L, SPC, L], F32R) for c in range(NCHUNK)}
        xp = sb("s_xp", [L, BPC, NPAD], F32R)
        cb = sb("s_cb", [L, BPC, H], F32R)
        wc = sb("s_wc", [L, L], F32R)
        dwt = sb("s_dwt", [H, H], F32)
        db = sb("s_db", [H, H], F32)
        ht = sb("s_ht", [H, BPC], F32)
        vb = sb("s_vb", [L, BPC, H], F32)
        ct = sb("s_ct", [L, BPC], F32)
        bia = sb("s_bia", [L, 2], F32)
        ones = sb("s_ones", [H, L], F32)
        g = sb("s_g_t", [H, BPC, H], F32)
        tb = sb("s_tb_t", [L, 1], F32)
        feats = sb("s_feats_t", [L, BPC, H], F32)
        prod = sb("s_prod", [L, BPC, H], F32)
        scores = sb("s_scores_t", [L, BPC], F32)
        e = sb("s_e_t", [L, BPC], F32)
        recip = sb("s_recip_t", [1, BPC], F32)
        recb = sb("s_recb", [L, BPC], F32)
        attn = sb("s_attn_t", [L, BPC], F32)
        ncov = sb("s_ncov_t", [L, BPC], F32)
        cxs = [sb(f"s_cxs{b}", [1, H], F32) for b in range(BPC)]

        acc = ps("p_acc", [L, BPC * H], F32)
        sums = ps("p_sums", [1, BPC], F32)
        cx = [ps(f"p_cx{b}", [1, H], F32) for b in range(BPC)]

        # one semaphore per DMA transfer: completions of back-to-back DMAs
        # on one HWDGE ring interleave their 16 sub-increments, so a shared
        # counting semaphore has no safe intermediate wait points.
        s_ch = {c: sem(f"s_ch{c}") for c in range(NCHUNK)}
        s_in = {n: sem(f"s_in_{n}")
                for n in ["xp", "cb", "wc", "dwt", "db", "ht", "vb", "ct",
                          "bia"]}
        s_g = sem("s_g")
        s_tb = sem("s_tb")
        s_acc = sem("s_acc")
        s_feats = sem("s_feats")
        s_scores = sem("s_scores")
        s_e = sem("s_e")
        s_sums = sem("s_sums")
        s_recip = sem("s_recip")
        s_bcast = sem("s_bcast")
        s_attn = sem("s_attn")
        s_ncov = sem("s_ncov")
        s_ctxmm = sem("s_ctxmm")
        s_cxs = sem("s_cxs")
        s_out = sem("s_out")
        s_chain = sem("s_chain")

        # chunk consumption order: chunk with s'=63 first; ring assignment
        # alternates sync/scalar in consumption order so arrivals match.
        chunk_order = [63 // SPC] + [c for c in range(NCHUNK) if c != 63 // SPC]
        ring_of = {}
        for i, c in enumerate(chunk_order):
            if i >= NCHUNK - 4:
                ring_of[c] = "C"          # SWDGE gets the last-consumed chunks
            else:
                ring_of[c] = "AB"[i % 2]

        with nc.Block() as block:

            @block.sync
            def _(sync):
                sync.dma_start(xp[:], d["xp"][:]).then_inc(s_in["xp"], 16)
                for c in chunk_order:
                    if ring_of[c] == "A":
                        sync.dma_start(
                            wch[c][:], d["w"][:, c * SPC:(c + 1) * SPC, :]
                        ).then_inc(s_ch[c], 16)
                # outputs
                sync.wait_ge(s_ncov, 1)
                sync.dma_start(d["attn_t"][:], attn[:]).then_inc(s_out, 16)
                sync.dma_start(d["ncov_t"][:], ncov[:]).then_inc(s_out, 16)
                sync.wait_ge(s_out, 64)

            @block.scalar
            def _(scalar):
                scalar.dma_start(cb[:], d["cb"][:]).then_inc(s_in["cb"], 16)
                scalar.dma_start(vb[:], d["vb"][:]).then_inc(s_in["vb"], 16)
                for c in chunk_order:
                    if ring_of[c] == "B":
                        scalar.dma_start(
                            wch[c][:], d["w"][:, c * SPC:(c + 1) * SPC, :]
                        ).then_inc(s_ch[c], 16)
                # tanh(acc + bias)
                scalar.wait_ge(s_acc, 1)
                scalar.wait_ge(s_tb, 1)
                nc.scalar.activation(
                    feats[:], acc[:].rearrange("o (b h) -> o b h", b=BPC),
                    mybir.ActivationFunctionType.Tanh, bias=tb[:], scale=1.0,
                ).then_inc(s_feats, 1)
                scalar.wait_ge(s_scores, 1)
                nc.scalar.activation(
                    e[:], scores[:], mybir.ActivationFunctionType.Exp
                ).then_inc(s_e, 1)
                for b in range(BPC):
                    scalar.wait_ge(s_cxs, b + 1)
                    scalar.dma_start(d["ctx"][b:b + 1, :], cxs[b][:]).then_inc(
                        s_out, 16)

            @block.gpsimd
            def _(gpsimd):
                gpsimd.dma_start(wc[:], d["wc"][:]).then_inc(s_in["wc"], 16)
                gpsimd.dma_start(dwt[:], d["dwt"][:]).then_inc(s_in["dwt"], 16)
                gpsimd.dma_start(db[:], d["db"][:]).then_inc(s_in["db"], 16)
                gpsimd.dma_start(ht[:], d["ht"][:]).then_inc(s_in["ht"], 16)
                gpsimd.dma_start(ct[:], d["ct"][:]).then_inc(s_in["ct"], 16)
                gpsimd.dma_start(bia[:], d["bia"][:]).then_inc(s_in["bia"], 16)
                for c in chunk_order:
                    if ring_of[c] == "C":
                        gpsimd.dma_start(
                            wch[c][:], d["w"][:, c * SPC:(c + 1) * SPC, :]
                        ).then_inc(s_ch[c], 16)
                gpsimd.wait_ge(s_recip, 1)
                nc.gpsimd.partition_broadcast(recb[:], recip[:]).then_inc(
                    s_bcast, 1)

            @block.vector
            def _(vector):
                vector.memset(ones[:], 1.0)
                vector.wait_ge(s_in["dwt"], 16)
                vector.wait_ge(s_in["db"], 16)
                vector.wait_ge(s_in["ht"], 16)
                for b in range(BPC):
                    ins = nc.vector.scalar_tensor_tensor(
                        out=g[:, b, :], in0=dwt[:], scalar=ht[:, b:b + 1],
                        in1=db[:],
                        op0=mybir.AluOpType.mult, op1=mybir.AluOpType.add,
                    )
                    if b == BPC - 1:
                        ins.then_inc(s_g, 1)
                vector.wait_ge(s_in["bia"], 16)
                nc.vector.tensor_add(
                    tb[:], bia[:, 0:1], bia[:, 1:2]).then_inc(s_tb, 1)
                vector.wait_ge(s_feats, 1)
                vector.wait_ge(s_in["vb"], 16)
                nc.vector.tensor_mul(prod[:], feats[:], vb[:]).then_inc(
                    s_chain, 1)
                vector.wait_ge(s_chain, 1)
                nc.vector.tensor_reduce(
                    scores[:], prod[:], axis=mybir.AxisListType.X,
                    op=mybir.AluOpType.add).then_inc(s_scores, 1)
                vector.wait_ge(s_sums, 1)
                nc.vector.reciprocal(recip[:], sums[:]).then_inc(s_recip, 1)
                vector.wait_ge(s_bcast, 1)
                nc.vector.tensor_mul(attn[:], e[:], recb[:]).then_inc(s_attn, 1)
                vector.wait_ge(s_attn, 1)
                vector.wait_ge(s_in["ct"], 16)
                nc.vector.tensor_add(ncov[:], attn[:], ct[:]).then_inc(s_ncov, 1)
                for b in range(BPC):
                    vector.wait_ge(s_ctxmm, b + 1)
                    nc.vector.tensor_copy(cxs[b][:], cx[b][:]).then_inc(
                        s_cxs, 1)

            @block.tensor
            def _(tensor):
                accv = acc[:].rearrange("o (b h) -> o b h", b=BPC)
                tensor.wait_ge(s_in["xp"], 16)
                first = True
                for c in chunk_order:
                    tensor.wait_ge(s_ch[c], 16)
                    shifts = list(range(c * SPC, (c + 1) * SPC))
                    if first:
                        shifts.remove(63)
                        shifts = [63] + shifts
                    for s in shifts:
                        nc.tensor.matmul(
                            accv, wch[c][:, s % SPC, :], xp[:, :, s:s + H],
                            start=first, stop=False,
                        )
                        first = False
                tensor.wait_ge(s_in["wc"], 16)
                tensor.wait_ge(s_in["cb"], 16)
                nc.tensor.matmul(accv, wc[:], cb[:], start=False, stop=False)
                tensor.wait_ge(s_g, 1)
                nc.tensor.matmul(
                    accv, ones[:], g[:], start=False, stop=True,
                ).then_inc(s_acc, 1)
                tensor.wait_ge(s_e, 1)
                nc.tensor.matmul(
                    sums[:], ones[:L, 0:1], e[:], start=True, stop=True,
                ).then_inc(s_sums, 1)
                tensor.wait_ge(s_attn, 1)
                for b in range(BPC):
                    nc.tensor.matmul(
                        cx[b][:], attn[:, b:b + 1],
                        xp[:, b, 63:63 + H].bitcast(F32),
                        start=True, stop=True,
                    ).then_inc(s_ctxmm, 1)

    nc.compile()
    return nc


_NC = None


def make_in_maps(hidden, encoder_outputs, coverage, attn_conv_w, attn_conv_b,
                 cvg_conv_w, cvg_conv_b, dec_w, dec_b, v):
    f = np.float32
    hidden = np.asarray(hidden, f)
    encoder_outputs = np.asarray(encoder_outputs, f)
    coverage = np.asarray(coverage, f)
    dec_w = np.asarray(dec_w, f)
    dec_b = np.asarray(dec_b, f)
    v = np.asarray(v, f)

    # shared tensors
    w_host = np.ascontiguousarray(
        np.transpose(np.asarray(attn_conv_w, f)[:, :, (H - 1) // 2, :], (1, 2, 0)))
    wc_host = np.ascontiguousarray(np.asarray(cvg_conv_w, f)[:, :, 0, (H - 1) // 2].T)
    dwt_host = np.ascontiguousarray(dec_w.T)
    db_host = np.ascontiguousarray(np.tile((dec_b / H)[None, :], (H, 1)))
    bia_host = np.ascontiguousarray(
        np.stack([np.asarray(attn_conv_b, f), np.asarray(cvg_conv_b, f)], axis=1))

    import ml_dtypes
    mmdt = {"f32r": np.float32, "f32": np.float32,
            "bf16": ml_dtypes.bfloat16}[MM_DT_NAME]
    w_host = w_host.astype(mmdt)
    wc_host = wc_host.astype(mmdt)
    in_maps = []
    for c in range(NCORES):
        bs = slice(c * BPC, (c + 1) * BPC)
        xT = np.transpose(encoder_outputs[bs], (1, 0, 2))        # [L, BPC, H]
        xp = np.zeros((L, BPC, NPAD), f)
        xp[:, :, (H - 1) // 2:(H - 1) // 2 + H] = xT
        covT = np.ascontiguousarray(coverage[bs].T)              # [L, BPC]
        in_maps.append({
            "w": w_host, "wc": wc_host, "dwt": dwt_host, "db": db_host,
            "bia": bia_host,
            "xp": np.ascontiguousarray(xp).astype(mmdt),
            "cb": np.ascontiguousarray(
                np.broadcast_to(covT[:, :, None], (L, BPC, H))).astype(mmdt),
            "ct": covT,
            "ht": np.ascontiguousarray(hidden[bs].T),
            "vb": np.ascontiguousarray(
                np.broadcast_to(v[bs][None, :, :], (L, BPC, H))),
        })
    return in_maps


def kernel(**inputs):
    global _NC, LAST_RESULTS
    _install_ntff_shim()
    if _NC is None:
        if os.environ.get("KERNEL_IMPL", "tile") == "raw":
            _NC = _build_raw()
        else:
            _NC = _build()
    in_maps = make_in_maps(**inputs)

    res = run_bass_kernel_spmd(
        _NC, in_maps, core_ids=list(range(NCORES)),
        trace=bool(int(os.environ.get("KERNEL_TRACE", "0"))),
    )
    LAST_RESULTS = res

    context = np.concatenate([r["ctx"] for r in res.results], axis=0)
    attn = np.concatenate([r["attn_t"].T for r in res.results], axis=0)
    ncov = np.concatenate([r["ncov_t"].T for r in res.results], axis=0)
    return context, attn, ncov


# revision 27
# speedup vs baseline: 1.2124x; 1.2124x over previous
"""Trainium2 Bass kernel for nn_AttnCalc (additive attention with coverage).

Math (see reference):
  B, L, H = 16, 96, 128
  enc_f[b,o,w] = conv2d(encoder_outputs as NCHW [B,L,1,H], W[L,L,H,H], same pad)
               = sum_{i,s'} W[o,i,63,s'] * x[b,i,w+s'-63]        (only kh=63 row survives)
  cvg_f[b,o]   = sum_i cvg_w[o,i,0,63] * coverage[b,i]           (only kw=63 col survives)
  dec_f[b,h]   = hidden @ dec_w.T + dec_b
  feats = tanh(enc_f + dec_f + cvg_f + enc_bias + cvg_bias)
  scores[b,l] = sum_h feats[b,l,h] * v[b,h];  attn = softmax_L(scores)
  context[b,h] = sum_l attn[b,l] * encoder_outputs[b,l,h]
  new_coverage = coverage + attn

Strategy: data-parallel over batch (2 per core, 8 cores, no collectives).
Per-core layout keeps L on SBUF partitions and (b, h/w) on the free dim:
the conv becomes 128 shift-matmuls accumulating into one PSUM tile
[96, 2*128], in float32r (TF32-class; measured max rel err ~1.5e-4).
"""

import os
import sys
import types

import numpy as np

sys.path.insert(0, "/opt/trn_rl_repo")

import concourse.bass as bass  # noqa: E402
import concourse.mybir as mybir  # noqa: E402
import concourse.tile as tile  # noqa: E402
from concourse import bacc  # noqa: E402
from concourse.bass_utils import run_bass_kernel_spmd  # noqa: E402

B, L, H = 16, 96, 128
NCORES = 8
BPC = B // NCORES          # batches per core
NPAD = 2 * H               # padded width (63 left + 128 + 65 right)
NCHUNK = 16                # weight DMA chunks
SPC = H // NCHUNK          # shifts per weight chunk

F32 = mybir.dt.float32
MM_DT_NAME = os.environ.get("MM_DT", "fp16")
F32R = {"f32r": mybir.dt.float32r,
        "bf16": mybir.dt.bfloat16,
        "fp16": mybir.dt.float16,
        "f32": mybir.dt.float32}[MM_DT_NAME]
BANDED = MM_DT_NAME in ("bf16", "fp16") and os.environ.get("BAND", "1") == "1"

# results of the last traced run (read by test.py)
LAST_RESULTS = None


def _install_ntff_shim():
    """antenv.axon_hooks is missing from this image; recreate it so
    run_bass_kernel_spmd(trace=True) can capture NTFF profiles."""
    if "antenv.axon_hooks" in sys.modules:
        return
    try:
        import antenv
    except ImportError:
        return
    mod = types.ModuleType("antenv.axon_hooks")
    mod._hook = None

    def set_axon_ntff_profile_hook(h):
        mod._hook = h

    def get_axon_ntff_profile_hook():
        return mod._hook

    mod.set_axon_ntff_profile_hook = set_axon_ntff_profile_hook
    mod.get_axon_ntff_profile_hook = get_axon_ntff_profile_hook
    sys.modules["antenv.axon_hooks"] = mod
    antenv.axon_hooks = mod
    try:
        from trn_agent_boot.trn_boot import _ntff_profile_via_ctypes

        set_axon_ntff_profile_hook(
            _ntff_profile_via_ctypes("/opt/axon/libaxon_pjrt.so")
        )
    except Exception:
        pass


def _build():
    nc = bacc.Bacc("TRN2", target_bir_lowering=False, debug=False)

    # weights/constants shared by all cores (each core gets the same array)
    w_d = nc.dram_tensor("w", [L, H, L], F32R, kind="ExternalInput")      # [i, s', o]
    wc_d = nc.dram_tensor("wc", [L, L], F32R, kind="ExternalInput")       # [i, o]
    dwt_d = nc.dram_tensor("dwt", [H, H], F32, kind="ExternalInput")      # dec_w.T [k, h]
    db_d = nc.dram_tensor("db", [H, H], F32, kind="ExternalInput")        # dec_b/H tiled [k, h]
    bia_d = nc.dram_tensor("bia", [L, 2], F32, kind="ExternalInput")      # attn_b | cvg_b

    # per-core activations
    xp_d = nc.dram_tensor("xp", [L, BPC, NPAD], F32R, kind="ExternalInput")
    cb_d = nc.dram_tensor("cb", [L, BPC, H], F32R, kind="ExternalInput")  # cov bcast
    ct_d = nc.dram_tensor("ct", [L, BPC], F32, kind="ExternalInput")      # coverage.T
    ht_d = nc.dram_tensor("ht", [H, BPC], F32, kind="ExternalInput")      # hidden.T
    vb_d = nc.dram_tensor("vb", [L, BPC, H], F32, kind="ExternalInput")   # v bcast

    at_o = nc.dram_tensor("attn_t", [L, BPC], F32, kind="ExternalOutput")
    nc_o = nc.dram_tensor("ncov_t", [L, BPC], F32, kind="ExternalOutput")
    cx_o = nc.dram_tensor("ctx", [BPC, H], F32, kind="ExternalOutput")

    NF = BPC * H  # psum free size

    with tile.TileContext(nc) as tc:
        with (
            tc.tile_pool(name="pool", bufs=1) as pool,
            tc.tile_pool(name="psum", bufs=1, space="PSUM") as pp,
        ):
            # --- loads: small inputs first, then the 4.7MB weight stream
            # spread across all engines' DMA queues so transfers overlap
            # with the PE shift-matmul stream.
            xp = pool.tile([L, BPC, NPAD], F32R)
            nc.sync.dma_start(xp[:], xp_d[:])
            cb = pool.tile([L, BPC, H], F32R)
            nc.scalar.dma_start(cb[:], cb_d[:])
            wc = pool.tile([L, L], F32R)
            nc.gpsimd.dma_start(wc[:], wc_d[:])
            dwt = pool.tile([H, H], F32)
            nc.gpsimd.dma_start(dwt[:], dwt_d[:])
            db = pool.tile([H, H], F32)
            nc.sync.dma_start(db[:], db_d[:])
            ht = pool.tile([H, BPC], F32)
            nc.gpsimd.dma_start(ht[:], ht_d[:])
            vb = pool.tile([L, BPC, H], F32)
            nc.scalar.dma_start(vb[:], vb_d[:])
            ct = pool.tile([L, BPC], F32)
            nc.scalar.dma_start(ct[:], ct_d[:])
            bia = pool.tile([L, 2], F32)
            nc.gpsimd.dma_start(bia[:], bia_d[:])

            dma_engines = [nc.sync, nc.scalar, nc.gpsimd]
            chunk_order = [63 // SPC] + [c for c in range(NCHUNK)
                                        if c != 63 // SPC]
            wch = {}
            for idx, c in enumerate(chunk_order):
                t = pool.tile([L, SPC, L], F32R, tag=f"w{c}")
                dma_engines[idx % len(dma_engines)].dma_start(
                    t[:], w_d[:, c * SPC:(c + 1) * SPC, :])
                wch[c] = t

            ones = pool.tile([H, L], F32)
            nc.vector.memset(ones[:], 1.0)

            # g[k, b, h] = dec_w.T[k, h] * hidden.T[k, b] + dec_b[h]/H
            g = pool.tile([H, BPC, H], F32)
            for b in range(BPC):
                nc.vector.scalar_tensor_tensor(
                    out=g[:, b, :], in0=dwt[:], scalar=ht[:, b:b + 1], in1=db[:],
                    op0=mybir.AluOpType.mult, op1=mybir.AluOpType.add,
                )

            # --- accumulate enc_f + cvg_f + dec_f in PSUM [L, BPC*H] ---
            # s'=63 first: it covers the full tile, so it carries start=True
            # and the rest can follow in weight-chunk streaming order.
            acc = pp.tile([L, BPC, H], F32)
            shift_order = [63] + [s for c in chunk_order
                                  for s in range(c * SPC, (c + 1) * SPC)
                                  if s != 63]
            for s in shift_order:
                nc.tensor.matmul(
                    acc[:],
                    wch[s // SPC][:, s % SPC, :],
                    xp[:, :, s:s + H],
                    start=(s == 63), stop=False,
                )
            nc.tensor.matmul(acc[:], wc[:], cb[:],
                             start=False, stop=False)
            nc.tensor.matmul(acc[:], ones[:], g[:],
                             start=False, stop=True)

            # --- epilogue ---
            tb = pool.tile([L, 1], F32)
            nc.vector.tensor_add(tb[:], bia[:, 0:1], bia[:, 1:2])
            feats = pool.tile([L, BPC, H], F32)
            nc.scalar.activation(
                feats[:], acc[:],
                mybir.ActivationFunctionType.Tanh, bias=tb[:], scale=1.0,
            )
            prod = pool.tile([L, BPC, H], F32)
            scores = pool.tile([L, BPC], F32)
            nc.vector.tensor_mul(prod[:], feats[:], vb[:])
            nc.vector.tensor_reduce(scores[:], prod[:],
                                    axis=mybir.AxisListType.X,
                                    op=mybir.AluOpType.add)
            e = pool.tile([L, BPC], F32)
            nc.scalar.activation(e[:], scores[:], mybir.ActivationFunctionType.Exp)

            sums = pp.tile([1, BPC], F32)
            nc.tensor.matmul(sums[:], ones[:L, 0:1], e[:], start=True, stop=True)
            recip = pool.tile([1, BPC], F32)
            nc.vector.reciprocal(recip[:], sums[:])
            recb = pool.tile([L, BPC], F32)
            nc.gpsimd.partition_broadcast(recb[:], recip[:])

            attn = pool.tile([L, BPC], F32)
            nc.vector.tensor_mul(attn[:], e[:], recb[:])
            ncov = pool.tile([L, BPC], F32)
            nc.vector.tensor_add(ncov[:], attn[:], ct[:])
            nc.sync.dma_start(at_o[:], attn[:])
            nc.sync.dma_start(nc_o[:], ncov[:])

            if MM_DT_NAME == "bf16":
                attn_mm = pool.tile([L, BPC], F32R)
                nc.vector.tensor_copy(attn_mm[:], attn[:])
            for b in range(BPC):
                cx = pp.tile([1, H], F32, tag=f"cx{b}")
                if MM_DT_NAME == "bf16":
                    nc.tensor.matmul(
                        cx[:], attn_mm[:, b:b + 1], xp[:, b, 63:63 + H],
                        start=True, stop=True,
                    )
                else:
                    nc.tensor.matmul(
                        cx[:], attn[:, b:b + 1],
                        xp[:, b, 63:63 + H].bitcast(F32),
                        start=True, stop=True,
                    )
                cxs = pool.tile([1, H], F32, tag=f"cxs{b}")
                nc.vector.tensor_copy(cxs[:], cx[:])
                nc.sync.dma_start(cx_o[b:b + 1, :], cxs[:])

    nc.compile()
    return nc


def _declare_drams(nc):
    d = {}
    d["w"] = nc.dram_tensor("w", [L, H, L], F32R, kind="ExternalInput")
    d["wc"] = nc.dram_tensor("wc", [L, L], F32R, kind="ExternalInput")
    d["dwt"] = nc.dram_tensor("dwt", [H, H], F32, kind="ExternalInput")
    d["db"] = nc.dram_tensor("db", [H, H], F32, kind="ExternalInput")
    d["bia"] = nc.dram_tensor("bia", [L, 2], F32, kind="ExternalInput")
    d["xp"] = nc.dram_tensor("xp", [L, BPC, NPAD], F32R, kind="ExternalInput")
    d["cb"] = nc.dram_tensor("cb", [L, BPC, H], F32R, kind="ExternalInput")
    d["ct"] = nc.dram_tensor("ct", [L, BPC], F32, kind="ExternalInput")
    d["ht"] = nc.dram_tensor("ht", [H, BPC], F32, kind="ExternalInput")
    d["vb"] = nc.dram_tensor("vb", [L, BPC, H], F32, kind="ExternalInput")
    d["attn_t"] = nc.dram_tensor("attn_t", [L, BPC], F32, kind="ExternalOutput")
    d["ncov_t"] = nc.dram_tensor("ncov_t", [L, BPC], F32, kind="ExternalOutput")
    d["ctx"] = nc.dram_tensor("ctx", [BPC, H], F32, kind="ExternalOutput")
    return d


def _build_raw():
    """Raw bacc (no Tile framework): hand-placed semaphores, no kernel-end
    barrier storm, DMA triggers issued from instruction 0."""
    from contextlib import ExitStack

    nc = bacc.Bacc("TRN2", target_bir_lowering=False, debug=False)
    d = _declare_drams(nc)

    with ExitStack() as ctx:
        def sb(name, shape, dt):
            return ctx.enter_context(nc.sbuf_tensor(name, list(shape), dt))

        def ps(name, shape, dt):
            return ctx.enter_context(nc.psum_tensor(name, list(shape), dt))

        def sem(name):
            return ctx.enter_context(nc.semaphore(name))

        wch = {c: sb(f"wch{c}", [L, SPC, L], F32R) for c in range(NCHUNK)}
        xp = sb("s_xp", [L, BPC, NPAD], F32R)
        cb = sb("s_cb", [L, BPC, H], F32R)
        wc = sb("s_wc", [L, L], F32R)
        dwt = sb("s_dwt", [H, H], F32)
        db = sb("s_db", [H, H], F32)
        ht = sb("s_ht", [H, BPC], F32)
        vb = sb("s_vb", [L, BPC, H], F32)
        ct = sb("s_ct", [L, BPC], F32)
        bia = sb("s_bia", [L, 2], F32)
        ones = sb("s_ones", [H, L], F32)
        g = sb("s_g_t", [H, BPC, H], F32)
        tb = sb("s_tb_t", [L, 1], F32)
        feats = sb("s_feats_t", [L, BPC, H], F32)
        prod = sb("s_prod", [L, BPC, H], F32)
        scores = sb("s_scores_t", [L, BPC], F32)
        e = sb("s_e_t", [L, BPC], F32)
        recip = sb("s_recip_t", [1, BPC], F32)
        recb = sb("s_recb", [L, BPC], F32)
        attn = sb("s_attn_t", [L, BPC], F32)
        attn_mm = sb("s_attn_mm", [L, BPC], F32R)
        ncov = sb("s_ncov_t", [L, BPC], F32)
        cxs = [sb(f"s_cxs{b}", [1, H], F32) for b in range(BPC)]

        acc = ps("p_acc", [L, BPC * H], F32)
        sums = ps("p_sums", [1, BPC], F32)
        cx = [ps(f"p_cx{b}", [1, H], F32) for b in range(BPC)]

        # one semaphore per DMA transfer: completions of back-to-back DMAs
        # on one HWDGE ring interleave their 16 sub-increments, so a shared
        # counting semaphore has no safe intermediate wait points.
        s_ch = {c: sem(f"s_ch{c}") for c in range(NCHUNK)}
        s_in = {n: sem(f"s_in_{n}")
                for n in ["xp", "cb", "wc", "dwt", "db", "ht", "vb", "ct",
                          "bia"]}
        s_g = sem("s_g")
        s_tb = sem("s_tb")
        s_acc = sem("s_acc")
        s_feats = sem("s_feats")
        s_scores = sem("s_scores")
        s_e = sem("s_e")
        s_sums = sem("s_sums")
        s_recip = sem("s_recip")
        s_bcast = sem("s_bcast")
        s_attn = sem("s_attn")
        s_ncov = sem("s_ncov")
        s_ctxmm = sem("s_ctxmm")
        s_cxs = sem("s_cxs")
        s_out = sem("s_out")
        s_chain = sem("s_chain")

        # chunk consumption order: chunk with s'=63 first; ring assignment
        # alternates sync/scalar in consumption order so arrivals match.
        chunk_order = [63 // SPC] + [c for c in range(NCHUNK) if c != 63 // SPC]
        ring_of = {}
        for i, c in enumerate(chunk_order):
            if i >= NCHUNK - 4:
                ring_of[c] = "C"          # SWDGE gets the last-consumed chunks
            else:
                ring_of[c] = "AB"[i % 2]

        with nc.Block() as block:

            @block.sync
            def _(sync):
                sync.dma_start(xp[:], d["xp"][:]).then_inc(s_in["xp"], 16)
                for c in chunk_order:
                    if ring_of[c] == "A":
                        sync.dma_start(
                            wch[c][:], d["w"][:, c * SPC:(c + 1) * SPC, :]
                        ).then_inc(s_ch[c], 16)
                # outputs
                sync.wait_ge(s_ncov, 1)
                sync.dma_start(d["attn_t"][:], attn[:]).then_inc(s_out, 16)
                sync.dma_start(d["ncov_t"][:], ncov[:]).then_inc(s_out, 16)
                sync.wait_ge(s_out, 64)

            @block.scalar
            def _(scalar):
                scalar.dma_start(cb[:], d["cb"][:]).then_inc(s_in["cb"], 16)
                scalar.dma_start(vb[:], d["vb"][:]).then_inc(s_in["vb"], 16)
                for c in chunk_order:
                    if ring_of[c] == "B":
                        scalar.dma_start(
                            wch[c][:], d["w"][:, c * SPC:(c + 1) * SPC, :]
                        ).then_inc(s_ch[c], 16)
                # tanh(acc + bias)
                scalar.wait_ge(s_acc, 1)
                scalar.wait_ge(s_tb, 1)
                nc.scalar.activation(
                    feats[:], acc[:].rearrange("o (b h) -> o b h", b=BPC),
                    mybir.ActivationFunctionType.Tanh, bias=tb[:], scale=1.0,
                ).then_inc(s_feats, 1)
                scalar.wait_ge(s_scores, 1)
                nc.scalar.activation(
                    e[:], scores[:], mybir.ActivationFunctionType.Exp
                ).then_inc(s_e, 1)
                for b in range(BPC):
                    scalar.wait_ge(s_cxs, b + 1)
                    scalar.dma_start(d["ctx"][b:b + 1, :], cxs[b][:]).then_inc(
                        s_out, 16)

            @block.gpsimd
            def _(gpsimd):
                gpsimd.dma_start(wc[:], d["wc"][:]).then_inc(s_in["wc"], 16)
                gpsimd.dma_start(dwt[:], d["dwt"][:]).then_inc(s_in["dwt"], 16)
                gpsimd.dma_start(db[:], d["db"][:]).then_inc(s_in["db"], 16)
                gpsimd.dma_start(ht[:], d["ht"][:]).then_inc(s_in["ht"], 16)
                gpsimd.dma_start(ct[:], d["ct"][:]).then_inc(s_in["ct"], 16)
                gpsimd.dma_start(bia[:], d["bia"][:]).then_inc(s_in["bia"], 16)
                for c in chunk_order:
                    if ring_of[c] == "C":
                        gpsimd.dma_start(
                            wch[c][:], d["w"][:, c * SPC:(c + 1) * SPC, :]
                        ).then_inc(s_ch[c], 16)
                gpsimd.wait_ge(s_recip, 1)
                nc.gpsimd.partition_broadcast(recb[:], recip[:]).then_inc(
                    s_bcast, 1)

            @block.vector
            def _(vector):
                vector.memset(ones[:], 1.0)
                vector.wait_ge(s_in["dwt"], 16)
                vector.wait_ge(s_in["db"], 16)
                vector.wait_ge(s_in["ht"], 16)
                for b in range(BPC):
                    ins = nc.vector.scalar_tensor_tensor(
                        out=g[:, b, :], in0=dwt[:], scalar=ht[:, b:b + 1],
                        in1=db[:],
                        op0=mybir.AluOpType.mult, op1=mybir.AluOpType.add,
                    )
                    if b == BPC - 1:
                        ins.then_inc(s_g, 1)
                vector.wait_ge(s_in["bia"], 16)
                nc.vector.tensor_add(
                    tb[:], bia[:, 0:1], bia[:, 1:2]).then_inc(s_tb, 1)
                vector.wait_ge(s_feats, 1)
                vector.wait_ge(s_in["vb"], 16)
                nc.vector.tensor_mul(prod[:], feats[:], vb[:]).then_inc(
                    s_chain, 1)
                vector.wait_ge(s_chain, 1)
                nc.vector.tensor_reduce(
                    scores[:], prod[:], axis=mybir.AxisListType.X,
                    op=mybir.AluOpType.add).then_inc(s_scores, 1)
                vector.wait_ge(s_sums, 1)
                nc.vector.reciprocal(recip[:], sums[:]).then_inc(s_recip, 1)
                vector.wait_ge(s_bcast, 1)
                nc.vector.tensor_mul(attn[:], e[:], recb[:]).then_inc(
                    s_chain, 1)
                vector.wait_ge(s_chain, 2)
                if mybir.dt.size(F32R) == 2:
                    nc.vector.tensor_copy(attn_mm[:], attn[:]).then_inc(
                        s_attn, 1)
                else:
                    nc.vector.engine_nop().then_inc(s_attn, 1)
                vector.wait_ge(s_attn, 1)
                vector.wait_ge(s_in["ct"], 16)
                nc.vector.tensor_add(ncov[:], attn[:], ct[:]).then_inc(s_ncov, 1)
                for b in range(BPC):
                    vector.wait_ge(s_ctxmm, b + 1)
                    nc.vector.tensor_copy(cxs[b][:], cx[b][:]).then_inc(
                        s_cxs, 1)

            @block.tensor
            def _(tensor):
                accv = acc[:].rearrange("o (b h) -> o b h", b=BPC)
                tensor.wait_ge(s_in["xp"], 16)
                first = True
                for c in chunk_order:
                    tensor.wait_ge(s_ch[c], 16)
                    shifts = list(range(c * SPC, (c + 1) * SPC))
                    if first:
                        shifts.remove(63)
                        shifts = [63] + shifts
                    for s in shifts:
                        if BANDED and not first:
                            dd = s - 63
                            w0 = max(0, -dd)
                            w1 = min(H, H - dd)
                            nc.tensor.matmul(
                                accv[:, :, w0:w1], wch[c][:, s % SPC, :],
                                xp[:, :, s + w0:s + w1],
                                start=False, stop=False,
                            )
                        else:
                            nc.tensor.matmul(
                                accv, wch[c][:, s % SPC, :], xp[:, :, s:s + H],
                                start=first, stop=False,
                            )
                        first = False
                tensor.wait_ge(s_in["wc"], 16)
                tensor.wait_ge(s_in["cb"], 16)
                nc.tensor.matmul(accv, wc[:], cb[:], start=False, stop=False)
                tensor.wait_ge(s_g, 1)
                nc.tensor.matmul(
                    accv, ones[:], g[:], start=False, stop=True,
                ).then_inc(s_acc, 1)
                tensor.wait_ge(s_e, 1)
                nc.tensor.matmul(
                    sums[:], ones[:L, 0:1], e[:], start=True, stop=True,
                ).then_inc(s_sums, 1)
                tensor.wait_ge(s_attn, 1)
                for b in range(BPC):
                    if mybir.dt.size(F32R) == 2:
                        nc.tensor.matmul(
                            cx[b][:], attn_mm[:, b:b + 1],
                            xp[:, b, 63:63 + H],
                            start=True, stop=True,
                        ).then_inc(s_ctxmm, 1)
                    else:
                        nc.tensor.matmul(
                            cx[b][:], attn[:, b:b + 1],
                            xp[:, b, 63:63 + H].bitcast(F32),
                            start=True, stop=True,
                        ).then_inc(s_ctxmm, 1)

    nc.compile()
    return nc


_NC = None


def make_in_maps(hidden, encoder_outputs, coverage, attn_conv_w, attn_conv_b,
                 cvg_conv_w, cvg_conv_b, dec_w, dec_b, v):
    f = np.float32
    hidden = np.asarray(hidden, f)
    encoder_outputs = np.asarray(encoder_outputs, f)
    coverage = np.asarray(coverage, f)
    dec_w = np.asarray(dec_w, f)
    dec_b = np.asarray(dec_b, f)
    v = np.asarray(v, f)

    # shared tensors
    w_host = np.ascontiguousarray(
        np.transpose(np.asarray(attn_conv_w, f)[:, :, (H - 1) // 2, :], (1, 2, 0)))
    wc_host = np.ascontiguousarray(np.asarray(cvg_conv_w, f)[:, :, 0, (H - 1) // 2].T)
    dwt_host = np.ascontiguousarray(dec_w.T)
    db_host = np.ascontiguousarray(np.tile((dec_b / H)[None, :], (H, 1)))
    bia_host = np.ascontiguousarray(
        np.stack([np.asarray(attn_conv_b, f), np.asarray(cvg_conv_b, f)], axis=1))

    import ml_dtypes
    mmdt = {"f32r": np.float32, "f32": np.float32,
            "fp16": np.float16,
            "bf16": ml_dtypes.bfloat16}[MM_DT_NAME]
    w_host = w_host.astype(mmdt)
    wc_host = wc_host.astype(mmdt)
    in_maps = []
    for c in range(NCORES):
        bs = slice(c * BPC, (c + 1) * BPC)
        xT = np.transpose(encoder_outputs[bs], (1, 0, 2))        # [L, BPC, H]
        xp = np.zeros((L, BPC, NPAD), f)
        xp[:, :, (H - 1) // 2:(H - 1) // 2 + H] = xT
        covT = np.ascontiguousarray(coverage[bs].T)              # [L, BPC]
        in_maps.append({
            "w": w_host, "wc": wc_host, "dwt": dwt_host, "db": db_host,
            "bia": bia_host,
            "xp": np.ascontiguousarray(xp).astype(mmdt),
            "cb": np.ascontiguousarray(
                np.broadcast_to(covT[:, :, None], (L, BPC, H))).astype(mmdt),
            "ct": covT,
            "ht": np.ascontiguousarray(hidden[bs].T),
            "vb": np.ascontiguousarray(
                np.broadcast_to(v[bs][None, :, :], (L, BPC, H))),
        })
    return in_maps


def kernel(**inputs):
    global _NC, LAST_RESULTS
    _install_ntff_shim()
    if _NC is None:
        if os.environ.get("KERNEL_IMPL", "tile") == "raw":
            _NC = _build_raw()
        else:
            _NC = _build()
    in_maps = make_in_maps(**inputs)

    res = run_bass_kernel_spmd(
        _NC, in_maps, core_ids=list(range(NCORES)),
        trace=bool(int(os.environ.get("KERNEL_TRACE", "0"))),
    )
    LAST_RESULTS = res

    context = np.concatenate([r["ctx"] for r in res.results], axis=0)
    attn = np.concatenate([r["attn_t"].T for r in res.results], axis=0)
    ncov = np.concatenate([r["ncov_t"].T for r in res.results], axis=0)
    return context, attn, ncov
